# revision 1
# baseline (speedup 1.0000x reference)
"""BlockTensorizedLinear Trainium2 kernel.

Computes y = x @ W^T + bias where W [2048, 2048] is assembled from
block-wise tensor-train factors:
  W^T[(j,n1,n2), (i,m1,m2)] = sum_r cores1[i,j,m1,n1,r] * cores2[i,j,r,m2,n2]

Strategy (8 NeuronCores, data-parallel over the 16384 batch*seq rows):
  * Host: transpose+block each core's 2048-row x shard; pack TT factor
    slices; replicate bias (in device column order).
  * Device (per core, identical SPMD program):
      1. W^T build: 128 small K=16 fp32r matmuls reconstruct the 16 MB
         W^T; a DVE merge + strided scatter-DMA through a DRAM scratch
         performs the TT patch permutation (out-dims out of the
         partition axis); each finished 128-row k-tile is reloaded into
         SBUF (resident for the whole GEMM).
      2. GEMM: per 128-row block, lhsT = x^T k-tile [128,128] fp32r
         (stationary), rhs = W^T[k] [128,512] (moving), 16 k-steps
         accumulate into 4 PSUM banks; DVE adds bias and restores
         standard column order; fat DMA out.
    fp32r (FP22 mantissa) runs the PE at bf16 speed with ~1e-4 relative
    accuracy. Scatter rides both HWDGE queues; x/y/reloads use SWDGE and
    the scalar queue so streams never block each other.
  * Host: concatenate the 8 row shards.
"""
import numpy as np

import concourse.bass as bass
import concourse.mybir as mybir
from concourse import bacc
from concourse.tile import TileContext
from concourse.bass_utils import run_bass_kernel_spmd

F32 = mybir.dt.float32
F32R = mybir.dt.float32r

B, S = 4, 4096
IN = OUT = 2048
NCORES = 8
ROWS = B * S // NCORES  # 2048 rows per core
KT = IN // 128          # 16 k-tiles
NT = OUT // 512         # 4 n-slices


def _build_device_kernel(tc, y, xt, g1t, g2t, bias_d, rows):
    nc = tc.nc
    RB = rows // 128
    xt = xt.bitcast(F32R)
    g1t = g1t.bitcast(F32R)
    g2t = g2t.bitcast(F32R)

    const = tc.alloc_tile_pool(name="const", bufs=1)
    g1 = const.tile([128, 4096], F32R, tag="g1")
    g2 = const.tile([128, 1024], F32R, tag="g2")
    bz = const.tile([128, 2048], F32, tag="bias")
    nc.scalar.dma_start(g2[:], g2t[:])
    for jj in range(4):
        nc.scalar.dma_start(g1[32 * jj:32 * (jj + 1), :],
                            g1t[32 * jj:32 * (jj + 1), :])
    nc.gpsimd.dma_start(bz[:], bias_d[:])
    wt = [const.tile([128, 2048], F32R, tag=f"wt{t}", name=f"wt{t}")
          for t in range(KT)]

    wpsum = tc.alloc_tile_pool(name="wpsum", bufs=2, space="PSUM")
    wstage = tc.alloc_tile_pool(name="wstage", bufs=2)
    wdpool = tc.alloc_tile_pool(name="wd", bufs=1, space="DRAM")
    wdram = wdpool.tile([IN, OUT], F32R, tag="wdram")
    # scatter target viewed as [t, c2, n1s, m1, n2, (i m2)]
    wdr = wdram.rearrange("(t c a b) (m w) -> t c a m b w",
                          t=KT, c=2, a=4, b=16, m=32, w=64)

    # ---- W^T build ----
    sc_eng = [nc.sync, nc.scalar]  # alternate scatter issue across HWDGE queues
    nsc = 0
    for t in range(KT):
        j, u = divmod(t, 4)
        st = wstage.tile([128, 2048], F32R, tag="wst", name=f"wst{t}")
        str_ = st.rearrange("p (c b i n) -> i p c b n", c=2, b=16, i=4, n=16)
        for i in range(4):
            ps = wpsum.tile([128, 512], F32, tag="wb", name=f"wb{t}_{i}")
            for c2 in range(2):
                c = 2 * u + c2
                lhsT = g1[32 * j:32 * j + 16,
                          i * 1024 + c * 128: i * 1024 + (c + 1) * 128]
                rhs = g2[32 * j:32 * j + 16, i * 256:(i + 1) * 256]
                nc.tensor.matmul(ps[:, c2 * 256:(c2 + 1) * 256], lhsT, rhs,
                                 start=True, stop=True,
                                 tile_position=(32 * j, 0))
            # merge i-chunk into stage (and round f32 -> f32r)
            nc.vector.tensor_copy(str_[i],
                                  ps.rearrange("p (c b n) -> p c b n",
                                               c=2, b=16, n=16))
        # scatter [p=(n1s,m1), f=(c2,n2,i,m2)] -> W^T rows in DRAM scratch;
        # per (c2, n1s) one 3-dim DMA iterating [m1, n2, (i m2)].
        for c2 in range(2):
            for n1s in range(4):
                src = st[32 * n1s:32 * (n1s + 1),
                         c2 * 1024:(c2 + 1) * 1024].rearrange(
                             "m (b w) -> m b w", b=16, w=64)
                sc_eng[nsc % 2].dma_start(wdr[t, c2, n1s], src)
                nsc += 1
        # reload the finished 128-row k-tile as a fat contiguous copy
        nc.gpsimd.dma_start(wt[t][:], wdram[t * 128:(t + 1) * 128, :])

    wstage.release()
    wpsum.release()

    # ---- GEMM ----
    xpool = tc.alloc_tile_pool(name="xp", bufs=2)
    ypool = tc.alloc_tile_pool(name="yp", bufs=1)
    gpsum = tc.alloc_tile_pool(name="gp", bufs=2, space="PSUM")

    KH = KT // 2  # load x in half-rowblock tiles to fit SBUF
    # host pre-blocked layout: rows (rb, h, p), cols (kk, c) -> fat 2D loads
    for rb in range(RB):
        xh = [xpool.tile([128, KH * 128], F32R, tag="xh", name=f"xh{rb}_{h}")
              for h in range(2)]
        for h in range(2):
            nc.gpsimd.dma_start(
                xh[h][:], xt[(rb * 2 + h) * 128:(rb * 2 + h + 1) * 128, :])
        pss = [gpsum.tile([128, 512], F32, tag=f"ps{n}", name=f"ps{n}_{rb}")
               for n in range(NT)]
        for k in range(KT):
            h, kk = divmod(k, KH)
            lhsT = xh[h][:, kk * 128:(kk + 1) * 128]
            for n in range(NT):
                nc.tensor.matmul(pss[n][:], lhsT,
                                 wt[k][:, n * 512:(n + 1) * 512],
                                 start=(k == 0), stop=(k == KT - 1))
        yb = ypool.tile([128, 2048], F32, tag="yb")
        # bias-add; dst AP restores standard column order i*512+m1*16+m2
        ybr = yb.rearrange("p (i m n) -> p m i n", i=4, m=32, n=16)
        for n in range(NT):
            nc.vector.tensor_add(
                ybr[:, 8 * n:8 * (n + 1)],
                pss[n].rearrange("p (m i w) -> p m i w", m=8, i=4, w=16),
                bz[:, n * 512:(n + 1) * 512].rearrange(
                    "p (m i w) -> p m i w", m=8, i=4, w=16))
        nc.scalar.dma_start(y[rb * 128:(rb + 1) * 128, :], yb[:])

    for pool in (gpsum, ypool, xpool, wdpool, const):
        pool.release()


_NC_CACHE = {}


def _get_nc():
    if "nc" not in _NC_CACHE:
        nc = bacc.Bacc("TRN2", debug=False, num_devices=NCORES)
        xt = nc.dram_tensor("xt", [2 * ROWS, 1024], F32, kind="ExternalInput")
        g1t = nc.dram_tensor("g1t", [128, 4096], F32, kind="ExternalInput")
        g2t = nc.dram_tensor("g2t", [128, 1024], F32, kind="ExternalInput")
        bias = nc.dram_tensor("bias", [128, 2048], F32, kind="ExternalInput")
        y = nc.dram_tensor("y", [ROWS, 2048], F32, kind="ExternalOutput")
        with TileContext(nc) as tc:
            _build_device_kernel(tc, y.ap(), xt.ap(), g1t.ap(), g2t.ap(),
                                 bias.ap(), ROWS)
        nc.compile()
        _NC_CACHE["nc"] = nc
    return _NC_CACHE["nc"]


def _host_prep_shared(cores1, cores2, bias):
    g1t = np.zeros((128, 4096), np.float32)
    g1t.reshape(4, 32, 4096)[:, :16, :] = (
        cores1.transpose(1, 4, 0, 3, 2).reshape(4, 16, 4096))
    g2t = np.zeros((128, 1024), np.float32)
    g2t.reshape(4, 32, 1024)[:, :16, :] = (
        cores2.transpose(1, 2, 0, 4, 3).reshape(4, 16, 1024))
    # bias in device column order m' = m1*64 + i*16 + m2
    bias_m = bias.reshape(4, 32, 16).transpose(1, 0, 2).reshape(OUT)
    bias2d = np.ascontiguousarray(
        np.broadcast_to(bias_m.reshape(1, OUT), (128, OUT)).astype(np.float32))
    return g1t, g2t, bias2d


def _host_xt_shard(x2d, core):
    """Block x^T so device loads are plain [128, 1024] contiguous tiles.

    Returns [4096, 1024]: rows (rb, h, p) = (row-block, k-half, in-sub),
    cols (kk, c) = (k-within-half, row-within-block)."""
    xs = x2d[core * ROWS:(core + 1) * ROWS]           # [2048 rows, 2048 in]
    arr = xs.reshape(ROWS // 128, 128, 2, 8, 128)     # [rb, c, h, kk, p]
    arr = arr.transpose(0, 2, 4, 3, 1)                # [rb, h, p, kk, c]
    return np.ascontiguousarray(arr.reshape(ROWS * 2, 1024))


def kernel(x, cores1, cores2, bias):
    x = np.ascontiguousarray(np.asarray(x, dtype=np.float32))
    cores1 = np.asarray(cores1, dtype=np.float32)
    cores2 = np.asarray(cores2, dtype=np.float32)
    bias = np.asarray(bias, dtype=np.float32)

    nc = _get_nc()
    g1t, g2t, bias2d = _host_prep_shared(cores1, cores2, bias)
    x2d = x.reshape(B * S, IN)
    in_maps = [{"xt": _host_xt_shard(x2d, c),
                "g1t": g1t, "g2t": g2t, "bias": bias2d}
               for c in range(NCORES)]

    res = run_bass_kernel_spmd(nc, in_maps, core_ids=list(range(NCORES)))
    y = np.concatenate([res.results[c]["y"] for c in range(NCORES)], axis=0)
    return y.reshape(B, S, OUT)
